# revision 26
# baseline (speedup 1.0000x reference)
"""Trainium2 Bass kernel for nn_CrossFrameAttention (sparse_attention).

Reference math per batch b:
    attn  = softmax_over_SHW(q @ K) + mask          (mask is per-key, query-independent)
    out   = attn @ V
which decomposes into  softmax(qK)V  +  (mask @ V)  where the second term is a
rank-1, query-independent bias handled on host.

Device strategy (8 NeuronCores): batch (2) x key-shard (4). Scores are computed
TRANSPOSED (keys on PSUM partitions, queries on the free axis) so that:
  - QK needs no transposes: lhsT = K tile [65 x 128], rhs = q [65 x 512]
  - the AV matmul consumes exp(scores) directly: lhsT = V tile [128 x 65]
  - softmax denominators come for free from a ones-column appended to V
  - a per-query numerical-stability shift enters as a 65th contraction row
    (keys gain a ones-row, queries gain a -mhat row)
All matmul operands are float32r: fp32 storage, PE truncates inputs to 12
mantissa bits and accumulates exactly in fp32 at 4x the plain-fp32 matmul
rate (measured; plain fp32 runs as 2 half-speed passes).
The shift mhat = max_k||k|| * ||q_n|| - 64 (Cauchy-Schwarz relaxed by 64 so
that no query's denominator can underflow to subnormals while the sum-exp
stays provably below fp32 max) is shared by all 4 key-shards of a batch, so
their partial (V^T P, sum P) results combine by plain addition on host;
normalization and the mask bias are tiny host ops.
"""

import numpy as np

import concourse.bacc as bacc
import concourse.mybir as mybir
import concourse.tile as tile
from concourse.bass_utils import run_bass_kernel_spmd

S, B, CK, CV, H, W = 8, 2, 64, 64, 64, 64
HW, SHW = H * W, S * H * W
N_CORES = 8
KEY_SHARDS = 4                 # key-parallel cores per batch
KC = SHW // KEY_SHARDS         # 8192 keys per core
NKT = KC // 128                # 64 key tiles of 128 keys
QCH = 512                      # queries per chunk (= one PSUM bank of fp32)
NQC = HW // QCH                # 8 query chunks
GROUP = 2                      # key tiles per PSUM score slot (= banks per slot)
SC_BUFS = 3                    # PSUM score slots
OUT_BUFS = 2                   # PSUM out-accumulator banks
SB_BUFS = 3                    # SBUF P-tile pool depth
SHIFT_RELAX = 64.0
RADIUS, WEIGHT = 0.1, 0.2

F32 = mybir.dt.float32
F32R = mybir.dt.float32r  # fp32 storage; PE truncates inputs to 12 mantissa
                          # bits and accumulates exactly, at 4x fp32 speed

_compiled_nc = None


def _key_groups():
    return [list(range(s, min(s + GROUP, NKT))) for s in range(0, NKT, GROUP)]


def _kernel_body(tc, keys, qry, vals, out, repeat=1):
    nc = tc.nc
    with (
        tc.tile_pool(name="persist", bufs=1) as persist,
        tc.tile_pool(name="p_pool", bufs=SB_BUFS) as p_pool,
        tc.tile_pool(name="o_pool", bufs=2) as o_pool,
        tc.tile_pool(name="ps_sc", bufs=SC_BUFS, space="PSUM") as ps_sc,
        tc.tile_pool(name="ps_out", bufs=OUT_BUFS, space="PSUM") as ps_out,
    ):
        keys_sb = persist.tile([CK + 1, KC], F32R)
        q_sb = persist.tile([CK + 1, HW], F32R)
        vals_sb = persist.tile([128, NKT * (CV + 1)], F32R)

        # chunked loads, first-needed-first so compute starts early: the first
        # QK group needs qry[:, :512] and keys[:, :256]; AV needs vals soon after
        def chunks(total, sizes):
            off = 0
            for s in sizes:
                yield off, min(s, total - off)
                off += s
                if off >= total:
                    break

        key_chunks = list(chunks(KC, [512, 512, 1024, 2048, 4096]))
        q_chunks = list(chunks(HW, [512, 1024, 2560]))
        val_chunks = list(chunks(NKT * (CV + 1), [260, 520, 1040, 2340]))
        dmas = [
            (q_sb, qry, q_chunks[0]),
            (keys_sb, keys, key_chunks[0]),
            (vals_sb, vals, val_chunks[0]),
            (keys_sb, keys, key_chunks[1]),
            (vals_sb, vals, val_chunks[1]),
            (q_sb, qry, q_chunks[1]),
            (keys_sb, keys, key_chunks[2]),
            (vals_sb, vals, val_chunks[2]),
            (keys_sb, keys, key_chunks[3]),
            (vals_sb, vals, val_chunks[3]),
            (q_sb, qry, q_chunks[2]),
            (keys_sb, keys, key_chunks[4]),
        ]
        for sb, dram, (off, w) in dmas:
            nc.sync.dma_start(out=sb[:, off:off + w], in_=dram[:, off:off + w])

        groups = _key_groups()
        for qi in range(NQC * repeat):
            qi = qi % NQC
            q_rhs = q_sb[:, qi * QCH:(qi + 1) * QCH]
            out_ps = ps_out.tile([CV + 1, QCH], F32)
            for g in groups:
                n = len(g) * QCH
                sc = ps_sc.tile([128, GROUP * QCH], F32, tag="sc")
                for j, kt in enumerate(g):
                    nc.tensor.matmul(
                        out=sc[:, j * QCH:(j + 1) * QCH],
                        lhsT=keys_sb[:, kt * 128:(kt + 1) * 128],
                        rhs=q_rhs,
                        start=True,
                        stop=True,
                    )
                p = p_pool.tile([128, GROUP * QCH], F32R, tag="p")
                nc.scalar.activation(
                    out=p[:, :n], in_=sc[:, :n],
                    func=mybir.ActivationFunctionType.Exp,
                )
                for j, kt in enumerate(g):
                    nc.tensor.matmul(
                        out=out_ps,
                        lhsT=vals_sb[:, kt * (CV + 1):(kt + 1) * (CV + 1)],
                        rhs=p[:, j * QCH:(j + 1) * QCH],
                        start=(kt == 0),
                        stop=(kt == NKT - 1),
                        skip_group_check=True,
                    )
            o_sb = o_pool.tile([CV + 1, QCH], F32)
            nc.vector.tensor_copy(out=o_sb, in_=out_ps)
            nc.sync.dma_start(out=out[:, qi * QCH:(qi + 1) * QCH], in_=o_sb)


def _build(repeat=1):
    nc = bacc.Bacc("TRN2", target_bir_lowering=False, debug=False, num_devices=N_CORES)
    keys = nc.dram_tensor("keys", [CK + 1, KC], F32R, kind="ExternalInput").ap()
    qry = nc.dram_tensor("qry", [CK + 1, HW], F32R, kind="ExternalInput").ap()
    vals = nc.dram_tensor("vals", [128, NKT * (CV + 1)], F32R, kind="ExternalInput").ap()
    out = nc.dram_tensor("out", [CV + 1, HW], F32, kind="ExternalOutput").ap()
    with tile.TileContext(nc) as tc:
        _kernel_body(tc, keys, qry, vals, out, repeat=repeat)
    nc.compile()
    return nc


def _get_compiled():
    global _compiled_nc
    if _compiled_nc is None:
        _compiled_nc = _build()
    return _compiled_nc


def _prep_inputs(mk, mv, qq):
    """Build the 8 per-core input dicts from the full fp32 arrays."""
    keys_f = mk.transpose(1, 2, 0, 3, 4).reshape(B, CK, SHW)     # [B, 64, 32768]
    vals_f = mv.transpose(1, 0, 3, 4, 2).reshape(B, SHW, CV)     # [B, 32768, 64]
    q_f = qq.reshape(B, CK, HW)                                  # [B, 64, 4096]

    # per-batch per-query stability shift (shared across the batch's key shards)
    mhat = np.empty((B, HW), np.float32)
    for b in range(B):
        maxk = np.sqrt(np.max((keys_f[b].astype(np.float64) ** 2).sum(0)))
        qn = np.sqrt((q_f[b].astype(np.float64) ** 2).sum(0))
        mhat[b] = (maxk * qn - SHIFT_RELAX).astype(np.float32)
    # round to 12 mantissa bits so the fp32r PE sees the shift row exactly
    m, e = np.frexp(mhat)
    mhat = np.ldexp(np.round(m * 4096.0) / 4096.0, e).astype(np.float32)

    in_maps = []
    for c in range(N_CORES):
        b, j = divmod(c, KEY_SHARDS)
        ksl = slice(j * KC, (j + 1) * KC)
        keys_aug = np.concatenate(
            [keys_f[b][:, ksl], np.ones((1, KC), np.float32)], axis=0
        )                                                         # [65, 8192]
        q_aug = np.concatenate([q_f[b], -mhat[b][None, :]], axis=0)  # [65, 4096]
        va = np.concatenate(
            [vals_f[b][ksl], np.ones((KC, 1), np.float32)], axis=1
        )                                                         # [8192, 65]
        vals_re = va.reshape(NKT, 128, CV + 1).transpose(1, 0, 2).reshape(128, -1)
        in_maps.append(
            {
                "keys": np.ascontiguousarray(keys_aug),
                "qry": np.ascontiguousarray(q_aug),
                "vals": np.ascontiguousarray(vals_re),
            }
        )
    return in_maps, vals_f


def kernel(memory_keys, memory_values, query_query, disparity, sequence_index):
    mk = np.asarray(memory_keys, dtype=np.float32)
    mv = np.asarray(memory_values, dtype=np.float32)
    qq = np.asarray(query_query, dtype=np.float32)
    dsp = np.asarray(disparity, dtype=np.float32)
    sqi = np.asarray(sequence_index)

    in_maps, vals_f = _prep_inputs(mk, mv, qq)
    nc = _get_compiled()
    res = run_bass_kernel_spmd(nc, in_maps, list(range(N_CORES))).results

    # host epilogue: combine shards, normalize, add the rank-1 mask bias
    idx = sqi.astype(np.float32)
    dist = np.sqrt((idx[:, :, 1] - 5.0) ** 2 + (idx[:, :, 0] - 5.0) ** 2)   # [B, S]
    total_disp = dist[:, :, None, None] * dsp                               # [B, S, H, W]
    weight = WEIGHT / S / H / W
    mask = np.where(np.abs(total_disp) > RADIUS, weight, 0.0).reshape(B, SHW)
    bias = np.einsum("bm,bmv->bv", mask.astype(np.float64), vals_f.astype(np.float64))

    out = np.empty((B, CV, H, W), np.float32)
    for b in range(B):
        acc = np.zeros((CV + 1, HW), np.float64)
        for j in range(KEY_SHARDS):
            acc += res[b * KEY_SHARDS + j]["out"]
        o = acc[:CV] / acc[CV] + bias[b][:, None]
        out[b] = o.astype(np.float32).reshape(CV, H, W)
    return out
